# revision 2
# baseline (speedup 1.0000x reference)
"""Trainium2 Bass kernel for KernelAttention (B=2, N=4096, C=512, H=8).

Sharding: 8 cores; core j handles batch b=j//4 and head-pair p=j%4
(heads 2p, 2p+1 -> a contiguous 128-column slice of the qkv/head space).
Each core computes q/k/v projections for its heads, full attention over
its batch, and a partial FC projection using only its heads' input
columns. The host sums the 4 partials per batch and adds the bias.

Device-side layout notes:
  - x arrives pre-transposed (xT [C, N]) so all matmuls have the
    contraction dim on partitions with no device-side transposes.
  - scores are computed transposed (s^T [j, i]) so the softmax
    numerator exp(scale*s) feeds the p@v matmul directly as the moving
    operand.  No max subtraction: |scale*s| <= ~2.5 for these inputs.
  - row sums come from an appended ones-column in v (psum row 64).
  - per-head normalization is applied after the per-head FC matmul via
    a per-partition tensor_scalar multiply, then the two heads are
    summed.
  - float32r matmuls (full PE rate at free dim 512).
"""

import numpy as np

B = 2
N = 4096
C = 512
H = 8
DH = 64
SCALE = C ** -0.5
NCORES = 8

ICHUNK = 512            # q rows per chunk
NCHUNK = N // ICHUNK    # 8
NJT = N // 128          # 32 j tiles
JBATCH = 2              # j tiles per exp batch (psum tile [128, 1024])
NJB = NJT // JBATCH     # 16

_BUILT = None


def _build():
    import concourse.tile as tile
    from concourse import bacc, mybir

    f32 = mybir.dt.float32
    f32r = mybir.dt.float32r
    bf16 = mybir.dt.bfloat16
    EXP = mybir.ActivationFunctionType.Exp

    nc = bacc.Bacc("TRN2", target_bir_lowering=False, debug=False,
                   num_devices=NCORES)

    xT = nc.dram_tensor("xT", [C, N], bf16, kind="ExternalInput").ap()
    wq = nc.dram_tensor("wq", [C, 128], bf16, kind="ExternalInput").ap()
    wk = nc.dram_tensor("wk", [C, 128], bf16, kind="ExternalInput").ap()
    wv = nc.dram_tensor("wv", [C, 128], bf16, kind="ExternalInput").ap()
    wfc = nc.dram_tensor("wfc", [128, C], f32r, kind="ExternalInput").ap()
    y = nc.dram_tensor("y", [N, C], f32, kind="ExternalOutput").ap()

    CO = C // 128  # 4 contraction subtiles for the projections

    from contextlib import ExitStack
    with tile.TileContext(nc) as tc, ExitStack() as ctx:
        const = ctx.enter_context(tc.tile_pool(name="const", bufs=1))
        ps_s = ctx.enter_context(tc.tile_pool(name="ps_s", bufs=2, space="PSUM"))
        ps_o = ctx.enter_context(tc.tile_pool(name="ps_o", bufs=2, space="PSUM"))
        ps_y = ctx.enter_context(tc.tile_pool(name="ps_y", bufs=2, space="PSUM"))
        pT_pool = ctx.enter_context(tc.tile_pool(name="pT", bufs=4))
        oT_pool = ctx.enter_context(tc.tile_pool(name="oT", bufs=2))
        sm_pool = ctx.enter_context(tc.tile_pool(name="small", bufs=2))
        tmp_pool = ctx.enter_context(tc.tile_pool(name="tmp", bufs=4))
        y_pool = ctx.enter_context(tc.tile_pool(name="ysb", bufs=3))

        # ---- constants / inputs to SBUF ----
        xT_sb = const.tile([128, CO, N], bf16)
        for co in range(CO):
            nc.sync.dma_start(xT_sb[:, co, :], xT[co * 128:(co + 1) * 128, :])
        wq_sb = const.tile([128, CO, 128], bf16)
        wk_sb = const.tile([128, CO, 128], bf16)
        wv_sb = const.tile([128, CO, 128], bf16)
        for w_sb, w_dram in ((wq_sb, wq), (wk_sb, wk), (wv_sb, wv)):
            for co in range(CO):
                nc.sync.dma_start(w_sb[:, co, :], w_dram[co * 128:(co + 1) * 128, :])
        wfc0_sb = const.tile([64, C], f32r)
        wfc1_sb = const.tile([64, C], f32r)
        nc.sync.dma_start(wfc0_sb[:], wfc[0:64, :])
        nc.sync.dma_start(wfc1_sb[:], wfc[64:128, :])
        ident = const.tile([128, 128], f32)
        from concourse.masks import make_identity
        make_identity(nc, ident)

        # ---- q/k/v projections ----
        # qT/kT: [dd=128 (2 heads x 64), i=N], via lhsT=w[:,co,:], rhs=xT
        qT_sb = const.tile([128, N], bf16)
        kT_sb = const.tile([128, N], bf16)
        for dst, w_sb in ((qT_sb, wq_sb), (kT_sb, wk_sb)):
            for ic2 in range(N // 1024):
                ps = ps_s.tile([128, 1024], f32, tag="s")
                for half in range(2):
                    isl = slice((ic2 * 2 + half) * 512, (ic2 * 2 + half + 1) * 512)
                    for co in range(CO):
                        nc.tensor.matmul(ps[:, half * 512:(half + 1) * 512],
                                         lhsT=w_sb[:, co, :],
                                         rhs=xT_sb[:, co, isl],
                                         start=(co == 0), stop=(co == CO - 1))
                nc.vector.tensor_copy(dst[:, ic2 * 1024:(ic2 + 1) * 1024], ps[:])

        # v natural layout + ones columns: vA [j=128, jt=32, 130]
        # cols 0:64 = v_h0, 64 = 1.0, 65:129 = v_h1, 129 = 1.0
        vA_sb = const.tile([128, NJT, 130], bf16)
        for jt8 in range(NJT // 8):
            ps = ps_s.tile([128, 1024], f32, tag="s")
            for s8 in range(8):
                jt = jt8 * 8 + s8
                for co in range(CO):
                    nc.tensor.matmul(ps[:, s8 * 128:(s8 + 1) * 128],
                                     lhsT=xT_sb[:, co, jt * 128:(jt + 1) * 128],
                                     rhs=wv_sb[:, co, :],
                                     start=(co == 0), stop=(co == CO - 1))
            # strided copy: [128, 8, 2, 64] view into the 130-col layout
            src = ps[:].rearrange("p (s8 h d) -> p s8 h d", s8=8, h=2)
            dst = vA_sb[:, jt8 * 8:(jt8 + 1) * 8, :].rearrange(
                "p s8 c -> p s8 c")  # keep AP; slice below
            for h in range(2):
                nc.vector.tensor_copy(
                    vA_sb[:, jt8 * 8:(jt8 + 1) * 8, h * 65:h * 65 + 64],
                    src[:, :, h, :])
        nc.vector.memset(vA_sb[:, :, 64:65], 1.0)
        nc.vector.memset(vA_sb[:, :, 129:130], 1.0)

        # ---- main attention + fc loop ----
        for ic in range(NCHUNK):
            isl = slice(ic * ICHUNK, (ic + 1) * ICHUNK)
            o_ps = [ps_o.tile([128, 512], f32, tag="o", name=f"o{ic}_{h}")
                    for h in range(2)]
            for jb in range(NJB):
                ps = {}
                pT = {}
                for t in range(JBATCH):
                    jt = jb * JBATCH + t
                    for h in range(2):
                        hp = slice(h * 64, (h + 1) * 64)
                        if t == 0:
                            ps[h] = ps_s.tile([128, 1024], f32, tag="s",
                                              name=f"s{ic}_{jb}_{h}")
                        nc.tensor.matmul(ps[h][:, t * 512:(t + 1) * 512],
                                         lhsT=kT_sb[hp, jt * 128:(jt + 1) * 128],
                                         rhs=qT_sb[hp, isl],
                                         start=True, stop=True)
                for h in range(2):
                    pT[h] = pT_pool.tile([128, 1024], bf16, tag="pT",
                                         name=f"pT{ic}_{jb}_{h}")
                    nc.scalar.activation(pT[h][:], ps[h][:], EXP, scale=SCALE)
                for h in range(2):
                    for t in range(JBATCH):
                        jt = jb * JBATCH + t
                        nc.tensor.matmul(o_ps[h][:65, :],
                                         lhsT=vA_sb[:, jt, h * 65:(h + 1) * 65],
                                         rhs=pT[h][:, t * 512:(t + 1) * 512],
                                         start=(jt == 0), stop=(jt == NJT - 1))

            # chunk tail: evacuate o psums, rowsum transposes, fc, normalize
            oT = []
            rs_sb = []
            for h in range(2):
                t = oT_pool.tile([64, 512], f32r, tag="oT", name=f"oT{ic}_{h}")
                nc.vector.tensor_copy(t[:], o_ps[h][0:64, :])
                r = sm_pool.tile([1, 512], f32, tag=f"rs{h}", name=f"rs{ic}_{h}")
                nc.vector.tensor_copy(r[:], o_ps[h][64:65, :])
                oT.append(t)
                rs_sb.append(r)
            rsP = ps_y.tile([128, 8], f32, tag="y", name=f"rsP{ic}")
            for sub in range(4):
                for h in range(2):
                    nc.tensor.transpose(rsP[:, sub * 2 + h:sub * 2 + h + 1],
                                        rs_sb[h][:, sub * 128:(sub + 1) * 128],
                                        ident[0:1, 0:1])
            rs_f = sm_pool.tile([128, 8], f32, tag="rsf", name=f"rsf{ic}")
            nc.vector.tensor_copy(rs_f[:], rsP[:])
            rcp = sm_pool.tile([128, 8], f32, tag="rcp", name=f"rcp{ic}")
            nc.vector.reciprocal(rcp[:], rs_f[:])

            for sub in range(4):
                y_ps = []
                for h in range(2):
                    yp = ps_y.tile([128, 512], f32, tag="y",
                                   name=f"y{ic}_{sub}_{h}")
                    nc.tensor.matmul(yp[:],
                                     lhsT=oT[h][:, sub * 128:(sub + 1) * 128],
                                     rhs=(wfc0_sb if h == 0 else wfc1_sb)[:],
                                     start=True, stop=True)
                    y_ps.append(yp)
                t1 = tmp_pool.tile([128, 512], f32, tag="t1",
                                   name=f"t1_{ic}_{sub}")
                nc.vector.tensor_scalar_mul(t1[:], y_ps[0][:],
                                            rcp[:, sub * 2:sub * 2 + 1])
                t2 = tmp_pool.tile([128, 512], f32, tag="t2",
                                   name=f"t2_{ic}_{sub}")
                nc.vector.tensor_scalar_mul(t2[:], y_ps[1][:],
                                            rcp[:, sub * 2 + 1:sub * 2 + 2])
                ysb = y_pool.tile([128, 512], f32, tag="ysb",
                                  name=f"ysb{ic}_{sub}")
                nc.vector.tensor_add(ysb[:], t1[:], t2[:])
                r0 = ic * ICHUNK + sub * 128
                nc.sync.dma_start(y[r0:r0 + 128, :], ysb[:])

    nc.compile()
    return nc


def _get_built():
    global _BUILT
    if _BUILT is None:
        _BUILT = _build()
    return _BUILT


def _make_in_maps(inputs):
    import ml_dtypes
    bf = ml_dtypes.bfloat16
    x = inputs["x"]
    w_qkv = inputs["w_qkv"]
    w_fc = inputs["w_fc"]
    in_maps = []
    for j in range(NCORES):
        b = j // 4
        p = j % 4
        cs = slice(p * 128, (p + 1) * 128)
        in_maps.append({
            "xT": np.ascontiguousarray(x[b].T).astype(bf),
            "wq": np.ascontiguousarray(w_qkv[cs, :].T).astype(bf),
            "wk": np.ascontiguousarray(w_qkv[C + p * 128:C + (p + 1) * 128, :].T).astype(bf),
            "wv": np.ascontiguousarray(w_qkv[2 * C + p * 128:2 * C + (p + 1) * 128, :].T).astype(bf),
            "wfc": np.ascontiguousarray(w_fc[:, cs].T),
        })
    return in_maps


def kernel(x, w_qkv, w_fc, b_fc):
    from concourse import bass_utils

    nc = _get_built()
    in_maps = _make_in_maps({"x": x, "w_qkv": w_qkv, "w_fc": w_fc})
    res = bass_utils.run_bass_kernel_spmd(nc, in_maps,
                                          core_ids=list(range(NCORES)))
    y = np.zeros((B, N, C), dtype=np.float32)
    for j in range(NCORES):
        y[j // 4] += res.results[j]["y"]
    y += b_fc.astype(np.float32)
    return y

